# revision 15
# baseline (speedup 1.0000x reference)
"""Trainium2 Bass kernel for 16-head causal MultiHeadAttention (S=4096, E=1024).

Sharding: tensor-parallel over heads across 8 NeuronCores. Each core receives
only its 128-row slice of hT (the transposed hidden states, shipped as bf16)
plus its per-head weight shards; a device-side AllGather reconstructs the full
hT in HBM, each core computes QKV projection for its 2 heads, flash-style
causal attention in scoresT layout ([t, s_q], softmax denominator via a
ones-column appended to V), and a partial out-projection over its 128 ctx
channels into an internal HBM buffer. After each 512-row s-block a device-side
ReduceScatter (fp32) sums that block's partials across cores — overlapping the
next block's compute — and the summed slice is cast to bf16 on the way to the
per-core output. Host<->device traffic is ~17 MB up / ~8.4 MB down instead of
~300 MB / ~134 MB for replicated fp32 I/O, which dominates end-to-end latency
over the axon-tunneled PJRT transport.

Matmuls run in bf16 with fp32 PSUM accumulation (rel err ~4e-3 vs the fp32
reference, well inside the 2e-2 gate). The causal mask tile is generated on
device with affine_select instead of being shipped in.

The module compiles the Bass program, jits the SPMD executor once, and runs a
zero-input warmup at import time, so kernel() itself is pure host prep +
transfer + execute (~0.6 s, of which ~0.55 s is tunnel wire time).
"""

import ml_dtypes
import numpy as np

import concourse.bacc as bacc
import concourse.mybir as mybir
from concourse.masks import make_identity
from concourse.tile import TileContext

N_CORES = 8
S = 4096
E = 1024
H = 16
D = 64
HPC = H // N_CORES          # heads per core = 2
C = HPC * D                 # ctx channels per core = 128
SCALE = 1.0 / np.sqrt(np.float32(E))  # note: sqrt(n_embd), per reference

SB = 512                    # s_q block (matmul free dim)
NSB = S // SB               # 8
TB = 128                    # t chunk (matmul contraction tile)
EB = 128                    # e chunk of the hidden dim
NEB = E // EB               # 8
NTB = S // TB               # 32
SLICE = S // N_CORES        # output rows per core after reduce-scatter = 512
NAG = 4                     # AllGather column chunks (overlap gather with QKV)
RSB = 1                     # s-blocks per ReduceScatter chunk

F32 = mybir.dt.float32
F32R = mybir.dt.float32r
BF16 = mybir.dt.bfloat16

_COMPILED = None
last_results = None  # test harness reads exec_time_ns off this


def _build():
    nc = bacc.Bacc(None, target_bir_lowering=False, num_devices=N_CORES)

    hTs = nc.declare_dram_parameter("hTs", [EB, S], BF16, isOutput=False)
    wq = nc.declare_dram_parameter("wq", [E, C], BF16, isOutput=False)
    wk = nc.declare_dram_parameter("wk", [E, C], BF16, isOutput=False)
    wv = nc.declare_dram_parameter("wv", [E, C], BF16, isOutput=False)
    bq = nc.declare_dram_parameter("bq", [1, C], BF16, isOutput=False)
    bk = nc.declare_dram_parameter("bk", [1, C], BF16, isOutput=False)
    bv = nc.declare_dram_parameter("bv", [1, C], BF16, isOutput=False)
    wo = nc.declare_dram_parameter("wo", [C, E], BF16, isOutput=False)
    y = nc.declare_dram_parameter("y", [SLICE, E], BF16, isOutput=True)

    with TileContext(nc) as tc:
        with (
            tc.tile_pool(name="dram", bufs=1, space="DRAM") as dram,
            tc.tile_pool(name="singles", bufs=1) as singles,
            tc.tile_pool(name="big", bufs=1) as big,
            tc.tile_pool(name="htp", bufs=18) as htp,
            tc.tile_pool(name="vtf", bufs=3) as vtf,
            tc.tile_pool(name="ep", bufs=8) as ep,
            tc.tile_pool(name="ef", bufs=3) as ef,
            tc.tile_pool(name="ip", bufs=3) as ip,
            tc.tile_pool(name="pqkv", bufs=1, space="PSUM") as pqkv,
            tc.tile_pool(name="pmix", bufs=1, space="PSUM") as pmix,
            tc.tile_pool(name="psc", bufs=3, space="PSUM") as psc,
            tc.tile_pool(name="pctx", bufs=1, space="PSUM") as pctx,
            tc.tile_pool(name="yp", bufs=4) as yp,
        ):
            # ---- gather the full hT from the per-core E-slices, in NAG
            # column chunks so the first s-blocks' QKV can start while the
            # rest of the gather is still in flight. Collective in/out APs
            # must be contiguous, so each chunk gets its own DRAM tile. ----
            GS = S // NAG
            ag_ins = [
                dram.tile([EB, GS], BF16, name=f"ag_in{g}") for g in range(NAG)
            ]
            hT_chunks = [
                dram.tile([E, GS], BF16, name=f"hT{g}") for g in range(NAG)
            ]
            for g in range(NAG):
                nc.sync.dma_start(
                    out=ag_ins[g][:],
                    in_=hTs[:, g * GS:(g + 1) * GS],
                )
                nc.gpsimd.collective_compute(
                    "AllGather",
                    mybir.AluOpType.bypass,
                    replica_groups=[list(range(N_CORES))],
                    ins=[ag_ins[g].opt()],
                    outs=[hT_chunks[g].opt()],
                )
            # partial out-projection accumulator (summed by ReduceScatter)
            y_part = dram.tile([S, E], F32)
            rs_out = dram.tile([SLICE, E], F32)

            # Weights, biases, constants
            wq_sb = singles.tile([EB, NEB, C], BF16)
            wk_sb = singles.tile([EB, NEB, C], BF16)
            wv_sb = singles.tile([EB, NEB, C], BF16)
            for w_dram, w_sb in ((wq, wq_sb), (wk, wk_sb), (wv, wv_sb)):
                nc.sync.dma_start(
                    out=w_sb[:], in_=w_dram.rearrange("(a p) m -> p a m", p=EB)
                )
            wo_sb = singles.tile([C, E], BF16)
            nc.sync.dma_start(out=wo_sb[:], in_=wo[:])
            bq_sb = singles.tile([1, C], BF16)
            bk_sb = singles.tile([1, C], BF16)
            bv_sb = singles.tile([1, C], BF16)
            nc.sync.dma_start(out=bq_sb[:], in_=bq[:])
            nc.sync.dma_start(out=bk_sb[:], in_=bk[:])
            nc.sync.dma_start(out=bv_sb[:], in_=bv[:])

            # causal boundary mask for the diagonal chunks: keep where p <= f
            mask_sb = singles.tile([TB, SB], F32)
            nc.vector.memset(mask_sb[:], 1.0)
            nc.gpsimd.affine_select(
                out=mask_sb[:],
                in_=mask_sb[:],
                compare_op=mybir.AluOpType.is_ge,
                fill=0.0,
                base=0,
                # keep where (-p + f) >= 0, i.e. p <= f
                pattern=[[1, SB]],
                channel_multiplier=-1,
            )

            ones_f = singles.tile([1, SB], F32)
            nc.vector.memset(ones_f[:], 1.0)
            ones_r = singles.tile([1, SB], BF16)
            nc.vector.tensor_copy(ones_r[:], ones_f[:])
            ones_col_f = singles.tile([TB, 1], F32)
            nc.vector.memset(ones_col_f[:], 1.0)
            ident = singles.tile([TB, TB], F32)
            make_identity(nc, ident[:])

            # Persistent activations
            qT_sb = big.tile([C, S], BF16)      # [c, s]
            kT_sb = big.tile([C, S], BF16)
            v_sb = big.tile([TB, NTB, 2 * (D + 1)], BF16)
            ctxT_sb = big.tile([C, S], BF16)

            ones64_f = singles.tile([1, D], F32)
            nc.vector.memset(ones64_f[:], 1.0)
            ones64_r = singles.tile([1, D], BF16)
            nc.vector.tensor_copy(ones64_r[:], ones64_f[:])

            for j in range(NSB):
                # ---- QKV projection for s-block j: three sequential
                # single-bank passes (q, k, v) over the held hT tiles ----
                hts = []
                for i in range(NEB):
                    ht = htp.tile([EB, SB], BF16)
                    hts.append(ht)
                    hT_g = hT_chunks[j * SB // GS]
                    sb0 = (j * SB) % GS
                    nc.sync.dma_start(
                        out=ht[:], in_=hT_g[i * EB:(i + 1) * EB, sb0:sb0 + SB]
                    )
                ps_q = pqkv.tile([C, SB], F32, tag="q")
                for i in range(NEB):
                    nc.tensor.matmul(
                        ps_q[:], wq_sb[:, i, :], hts[i][:], start=(i == 0), stop=False
                    )
                nc.tensor.matmul(ps_q[:], bq_sb[:], ones_r[:], start=False, stop=True)
                nc.vector.tensor_copy(qT_sb[:, j * SB:(j + 1) * SB], ps_q[:])
                ps_k = pqkv.tile([C, SB], F32, tag="q")
                for i in range(NEB):
                    nc.tensor.matmul(
                        ps_k[:], wk_sb[:, i, :], hts[i][:], start=(i == 0), stop=False
                    )
                nc.tensor.matmul(ps_k[:], bk_sb[:], ones_r[:], start=False, stop=True)
                nc.vector.tensor_copy(kT_sb[:, j * SB:(j + 1) * SB], ps_k[:])
                ps_v = pqkv.tile([C, SB], F32, tag="q")
                for i in range(NEB):
                    nc.tensor.matmul(
                        ps_v[:], wv_sb[:, i, :], hts[i][:], start=(i == 0), stop=False
                    )
                nc.tensor.matmul(ps_v[:], bv_sb[:], ones_r[:], start=False, stop=True)
                vt_f = vtf.tile([C, SB], F32)
                nc.vector.tensor_copy(vt_f[:], ps_v[:])
                for tb in range(SB // TB):
                    ic = j * (SB // TB) + tb  # global t-chunk id
                    ps_t = pmix.tile([TB, TB], F32, tag="tr")
                    nc.tensor.transpose(ps_t[:], vt_f[:, tb * TB:(tb + 1) * TB], ident[:])
                    for h in range(HPC):
                        base = h * (D + 1)
                        nc.vector.tensor_copy(
                            v_sb[:, ic, base:base + D], ps_t[:, h * D:(h + 1) * D]
                        )
                        nc.vector.tensor_copy(
                            v_sb[:, ic, base + D:base + D + 1], ones_col_f[:]
                        )

                # ---- causal attention for s-block j (both heads) ----
                nchunks = (j + 1) * (SB // TB)
                for h in range(HPC):
                    hp = h * D
                    vb = h * (D + 1)
                    ps_ctx = pctx.tile([D + 1, SB], F32, tag="ctx")
                    for i in range(nchunks):
                        ps_sc = psc.tile([TB, SB], F32, tag="sc")
                        et = ep.tile([TB, SB], BF16, tag="et")
                        diag = i - j * (SB // TB)
                        # Columns f < 128*diag of a diagonal chunk are fully
                        # masked; skip them in scores/exp/mask/PV entirely.
                        off = TB * diag if diag > 0 else 0
                        w = SB - off
                        nc.tensor.matmul(
                            ps_sc[:, off:SB],
                            kT_sb[hp:hp + D, i * TB:(i + 1) * TB],
                            qT_sb[hp:hp + D, j * SB + off:(j + 1) * SB],
                            start=True, stop=True,
                        )
                        if diag >= 0:  # chunk straddling the causal boundary
                            et_f = ef.tile([TB, SB], F32, tag="etf")
                            nc.scalar.activation(
                                out=et_f[:, off:SB], in_=ps_sc[:, off:SB],
                                func=mybir.ActivationFunctionType.Exp, scale=float(SCALE),
                            )
                            nc.vector.tensor_mul(
                                et[:, off:SB], et_f[:, off:SB], mask_sb[:, 0:w]
                            )
                        else:
                            nc.scalar.activation(
                                out=et[:], in_=ps_sc[:],
                                func=mybir.ActivationFunctionType.Exp, scale=float(SCALE),
                            )
                        nc.tensor.matmul(
                            ps_ctx[:, off:SB],
                            v_sb[:, i, vb:vb + D + 1],
                            et[:, off:SB],
                            start=(i == 0), stop=(i == nchunks - 1),
                        )
                    # normalize: ctxT = ctx_hat / denom (denom = row D of ps_ctx)
                    ctx_f = ip.tile([D + 1, SB], F32, tag="ctxf")
                    nc.vector.tensor_copy(ctx_f[:], ps_ctx[:])
                    inv_f = ip.tile([1, SB], F32, tag="invf")
                    nc.vector.reciprocal(inv_f[:], ctx_f[D:D + 1, :])
                    inv_r = ip.tile([1, SB], BF16, tag="invr")
                    nc.vector.tensor_copy(inv_r[:], inv_f[:])
                    ps_in = pmix.tile([D, SB], F32, tag="inv")
                    nc.tensor.matmul(ps_in[:], ones64_r[:], inv_r[:], start=True, stop=True)
                    inv64 = ip.tile([D, SB], F32, tag="inv64")
                    nc.vector.tensor_copy(inv64[:], ps_in[:])
                    nc.vector.tensor_mul(
                        ctxT_sb[hp:hp + D, j * SB:(j + 1) * SB],
                        ctx_f[0:D, :],
                        inv64[:],
                    )

                # ---- partial out-projection for s-block j ----
                for tb in range(SB // TB):
                    sb = j * (SB // TB) + tb
                    for eh in range(E // SB):
                        ps_o = pmix.tile([TB, SB], F32, tag="y")
                        nc.tensor.matmul(
                            ps_o[:],
                            ctxT_sb[:, sb * TB:(sb + 1) * TB],
                            wo_sb[:, eh * SB:(eh + 1) * SB],
                            start=True, stop=True,
                        )
                        y_t = yp.tile([TB, SB], F32, tag="yt")
                        nc.vector.tensor_copy(y_t[:], ps_o[:])
                        nc.sync.dma_start(
                            out=y_part[sb * TB:(sb + 1) * TB, eh * SB:(eh + 1) * SB],
                            in_=y_t[:],
                        )

                # ---- reduce-scatter finished partials across cores every
                # RSB blocks; overlaps with the next blocks' compute ----
                if (j + 1) % RSB == 0:
                    g = j // RSB
                    gs = RSB * (SLICE // NSB)  # output rows per RS group
                    rs_j = rs_out[g * gs:(g + 1) * gs, :]
                    nc.gpsimd.collective_compute(
                        "ReduceScatter",
                        mybir.AluOpType.add,
                        replica_groups=[list(range(N_CORES))],
                        ins=[y_part[g * RSB * SB:(g + 1) * RSB * SB, :].opt()],
                        outs=[rs_j.opt()],
                    )
                    nc.gpsimd.dma_start(
                        out=y[g * gs:(g + 1) * gs, :],
                        in_=rs_j,
                    )


    nc.compile()
    return nc


class _Exec:
    """One-time-compiled SPMD executor for the bass module.

    ``run_bass_kernel_spmd`` re-traces (and re-compiles) its jit wrapper on
    every call because the wrapped closure is recreated each time; compiling
    once here and caching keeps repeat calls at pure transfer+execute cost,
    and lets the import-time warmup absorb the compile.
    """

    def __init__(self, nc):
        import jax
        from concourse import bass2jax
        from jax.experimental.shard_map import shard_map
        from jax.sharding import Mesh, PartitionSpec

        bass2jax.install_neuronx_cc_hook()
        self.nc = nc
        self._donors = None
        partition_name = (
            nc.partition_id_tensor.name if nc.partition_id_tensor else None
        )
        in_names, out_names, out_avals, zero_templates = [], [], [], []
        for alloc in nc.m.functions[0].allocations:
            if not isinstance(alloc, mybir.MemoryLocationSet):
                continue
            name = alloc.memorylocations[0].name
            if alloc.kind == "ExternalInput":
                if name != partition_name:
                    in_names.append(name)
            elif alloc.kind == "ExternalOutput":
                out_names.append(name)
                shape = tuple(alloc.tensor_shape)
                dtype = mybir.dt.np(alloc.dtype)
                out_avals.append(jax.core.ShapedArray(shape, dtype))
                zero_templates.append((shape, dtype))
        self.in_names = list(in_names)
        self.out_names = list(out_names)
        self.zero_templates = zero_templates
        n_params = len(in_names)
        all_names = in_names + out_names
        if partition_name is not None:
            all_names = all_names + [partition_name]

        def _body(*args):
            operands = list(args)
            if partition_name is not None:
                operands.append(bass2jax.partition_id_tensor())
            outs = bass2jax._bass_exec_p.bind(
                *operands,
                out_avals=tuple(out_avals),
                in_names=tuple(all_names),
                out_names=tuple(out_names),
                lowering_input_output_aliases=(),
                sim_require_finite=True,
                sim_require_nnan=True,
                nc=nc,
            )
            return tuple(outs)

        devices = jax.devices()[:N_CORES]
        mesh = Mesh(np.array(devices), ("core",))
        n_outs = len(out_names)
        self.sharded = jax.jit(
            shard_map(
                _body,
                mesh=mesh,
                in_specs=(PartitionSpec("core"),) * (n_params + n_outs),
                out_specs=(PartitionSpec("core"),) * n_outs,
                check_rep=False,
            ),
            donate_argnums=tuple(range(n_params, n_params + n_outs)),
            keep_unused=True,
        )

    def warm(self, in_maps):
        arrs = self._dispatch(in_maps)
        for a in arrs:
            a.block_until_ready()
        self._donors = list(arrs)

    def run(self, in_maps):
        arrs = self._dispatch(in_maps)
        outs = [np.asarray(a) for a in arrs]
        self._donors = list(arrs)
        return outs

    def _dispatch(self, in_maps):
        concat_in = [
            np.concatenate([in_maps[c][name] for c in range(N_CORES)], axis=0)
            for name in self.in_names
        ]
        # The kernel writes every output element, so the donated buffers only
        # need the right shape/sharding: recycle the previous call's
        # device-resident outputs instead of uploading fresh zero buffers.
        donors = self._donors
        self._donors = None
        if donors is None:
            donors = [
                np.zeros((N_CORES * s[0],) + s[1:], d)
                for s, d in self.zero_templates
            ]
        return self.sharded(*concat_in, *donors)


_EXEC = None


def _setup():
    global _COMPILED, _EXEC
    if _EXEC is not None:
        return
    _COMPILED = _build()
    _EXEC = _Exec(_COMPILED)
    # Warmup: compile the jit wrapper, load the NEFF on all 8 cores, and
    # establish the collective comm once, off the measured path.
    zero_maps = [
        {
            "hTs": np.zeros((EB, S), ml_dtypes.bfloat16),
            "wq": np.zeros((E, C), ml_dtypes.bfloat16),
            "wk": np.zeros((E, C), ml_dtypes.bfloat16),
            "wv": np.zeros((E, C), ml_dtypes.bfloat16),
            "bq": np.zeros((1, C), ml_dtypes.bfloat16),
            "bk": np.zeros((1, C), ml_dtypes.bfloat16),
            "bv": np.zeros((1, C), ml_dtypes.bfloat16),
            "wo": np.zeros((C, E), ml_dtypes.bfloat16),
        }
        for _ in range(N_CORES)
    ]
    # Twice: the first warmup donates host zero buffers, the second donates
    # the first's device-resident outputs — the same argument signature real
    # calls use, so they hit the compiled-call cache.
    _EXEC.warm(zero_maps)
    _EXEC.warm(zero_maps)


try:
    _setup()
except Exception:  # fall back to lazy init inside kernel()
    _COMPILED = None
    _EXEC = None


def kernel(hidden_states, qkv_w, qkv_b, out_w, out_b):
    global last_results
    _setup()

    bf16 = ml_dtypes.bfloat16
    hT = np.ascontiguousarray(np.asarray(hidden_states).T.astype(bf16))
    wr = np.asarray(qkv_w).astype(bf16).reshape(E, H, 3, D)
    br = np.asarray(qkv_b).astype(bf16).reshape(H, 3, D)
    wor = np.asarray(out_w).astype(bf16).reshape(H, D, E)

    in_maps = []
    for c in range(N_CORES):
        heads = [HPC * c + h for h in range(HPC)]
        in_maps.append({
            "hTs": hT[c * EB:(c + 1) * EB],
            "wq": np.ascontiguousarray(wr[:, heads, 0, :].reshape(E, C)),
            "wk": np.ascontiguousarray(wr[:, heads, 1, :].reshape(E, C)),
            "wv": np.ascontiguousarray(wr[:, heads, 2, :].reshape(E, C)),
            "bq": np.ascontiguousarray(br[heads, 0, :].reshape(1, C)),
            "bk": np.ascontiguousarray(br[heads, 1, :].reshape(1, C)),
            "bv": np.ascontiguousarray(br[heads, 2, :].reshape(1, C)),
            "wo": np.ascontiguousarray(wor[heads].reshape(C, E)),
        })

    outs = _EXEC.run(in_maps)
    last_results = None
    yg = outs[_EXEC.out_names.index("y")].reshape(
        N_CORES, NSB // RSB, RSB * (SLICE // NSB), E
    )
    out = np.ascontiguousarray(yg.transpose(1, 0, 2, 3).reshape(S, E)).astype(np.float32)
    out += np.asarray(out_b, dtype=np.float32)
    return out

